# revision 12
# baseline (speedup 1.0000x reference)
"""Multi-head causal self-attention with RoPE on 8 Trainium2 NeuronCores.

Sharding: 12 heads over 8 cores. Core pairs (2p, 2p+1) share 3 heads:
  core 2p:   slot A = head 3p   (all 8 q-blocks), slot B = head 3p+1, q-blocks {0,1,6,7}
  core 2p+1: slot A = head 3p+2 (all 8 q-blocks), slot B = head 3p+1, q-blocks {2,3,4,5}
Every core: 2 heads on 128 partitions, 12 (head, q-block) pairs, identical causal cost.
Even/odd cores need different programs (q-block sets differ) -> two NEFFs dispatched
concurrently on device groups [0..3] / [4..7].

Pipeline (per core):
  x^T (f16) -> Q^T/K^T proj (f16 matmul, f32 PSUM) -> RoPE (DVE/Pool) -> f16 Q^T/K^T
  V computed directly as [token, dh] (x tiles stationary) -> bf16 [V_A|1|V_B|1] tiles
  logits^T = K_tile @ Q^T (f16) -> f32 PSUM -> exp (Act, scale=1/8) -> bf16 scores
  AV flipped: scores as stationary [keys,128q], moving [V_s|1] (65 cols) ->
    psAV [q, 4*(64+1)] f32 accumulated over key tiles; sums in col 64 of each chunk
  normalize: reciprocal (DVE) + per-partition tensor_scalar (Pool) -> bf16 A [q, dh]
  A^T via DMA xbar transpose [128,128] -> at_all bf16 [2*dh, S]
  O-proj: wo (bf16) x at_all -> f32 PSUM -> f16 SBUF -> DMA partials; host sums.

RoPE pairs are (d, d+16) within each 32-partition quadrant (folded into a host-side
weight-row permutation) so the pair swap is one stream_shuffle rotate-16.
"""
import sys, os
sys.path.insert(0, "/opt/trn_rl_repo")
os.environ.setdefault("MYCRO_LOCAL_CACHE", "1")

import numpy as np

S, D, H, DH = 4096, 768, 12, 64
NCH, CH = 8, 512     # token chunks (projection phase)
NQB, QB = 8, 512     # query blocks
NKT, KT = 32, 128    # key tiles
VPW = 130            # vp_all per-ktile width: [V_A(64) | 1 | V_B(64) | 1]
THETA = 10000.0
ROT16 = [(i + 16) % 32 for i in range(32)]

BSET_EVEN = (0, 1, 6, 7)
BSET_ODD = (2, 3, 4, 5)

# core -> (headA, headB)
CORE_HEADS = []
for p in range(4):
    CORE_HEADS.append((3 * p, 3 * p + 1))
    CORE_HEADS.append((3 * p + 2, 3 * p + 1))

# row r (0..63) inside a head slot -> original within-head dim.
# quadrant q = r//32, i = r%32: freq f = 16*q + (i%16); i<16 -> dim 2f, else 2f+1.
PERM64 = []
for r in range(64):
    q, i = r // 32, r % 32
    f = 16 * q + (i % 16)
    PERM64.append(2 * f if i < 16 else 2 * f + 1)
PERM64 = np.array(PERM64)

# psAV chunk base offsets: slot 0 chunks at 65*j in bank 0, slot 1 at 512+65*j
# in bank 1 (each 65-wide [A(64)|sum] region must stay within one 2KB bank).
AVB = [[65 * j for j in range(4)], [512 + 65 * j for j in range(4)]]

_PROGRAMS = {}


def _build_program(bset):
    import concourse.bass as bass
    import concourse.tile as tile
    from concourse import bacc, mybir
    from concourse.alu_op_type import AluOpType

    dt = mybir.dt
    F32, F16, BF16 = dt.float32, dt.float16, dt.bfloat16
    AF = mybir.ActivationFunctionType

    nc = bacc.Bacc("TRN2", target_bir_lowering=False, debug=False, num_devices=4)

    xt_d = nc.dram_tensor("xt", [D, S], F16, kind="ExternalInput").ap()
    wq_d = nc.dram_tensor("wq", [128, D], F16, kind="ExternalInput").ap()
    wk_d = nc.dram_tensor("wk", [128, D], F16, kind="ExternalInput").ap()
    wv_d = nc.dram_tensor("wv", [128, D], F16, kind="ExternalInput").ap()
    wot_d = nc.dram_tensor("wot", [128, D], BF16, kind="ExternalInput").ap()
    cosf_d = nc.dram_tensor("cosf", [128, S], F32, kind="ExternalInput").ap()
    sins_d = nc.dram_tensor("sins", [128, S], F32, kind="ExternalInput").ap()
    tri_d = nc.dram_tensor("tri", [128, 128], BF16, kind="ExternalInput").ap()
    opart_d = nc.dram_tensor("opart", [D, S], F16, kind="ExternalOutput").ap()
    DEBUG = bool(os.environ.get("MHA_DEBUG"))
    if DEBUG:
        dbg_kt = nc.dram_tensor("dbg_kt", [128, S], F32, kind="ExternalOutput").ap()
        dbg_qt = nc.dram_tensor("dbg_qt", [128, S], F32, kind="ExternalOutput").ap()
        dbg_vp = nc.dram_tensor("dbg_vp", [128, NKT * VPW], F32,
                                kind="ExternalOutput").ap()
        dbg_at = nc.dram_tensor("dbg_at", [128, S], F32, kind="ExternalOutput").ap()
        dbg_av = nc.dram_tensor("dbg_av", [128, 772], F32, kind="ExternalOutput").ap()
        dbg_st = nc.dram_tensor("dbg_st", [128, 512], F32, kind="ExternalOutput").ap()

    with tile.TileContext(nc) as tc:
        with (
            tc.tile_pool(name="const", bufs=1) as cp,
            tc.tile_pool(name="xc", bufs=2) as xcp,
            tc.tile_pool(name="rt", bufs=2) as rtp,
            tc.tile_pool(name="ex", bufs=3) as exp_pool,
            tc.tile_pool(name="stg", bufs=3) as stp,
            tc.tile_pool(name="rcp", bufs=2) as rcp,
            tc.tile_pool(name="osb", bufs=2) as osb,
            tc.tile_pool(name="psP", bufs=2, space="PSUM") as psP,
            tc.tile_pool(name="psL", bufs=2, space="PSUM") as psL,
            tc.tile_pool(name="psAV", bufs=1, space="PSUM") as psAV,
        ):
            kt_rot = cp.tile([128, S], F16, tag="ktrot")
            qt_rot = cp.tile([128, S], F16, tag="qtrot")
            vp_all = cp.tile([128, NKT * VPW], BF16, tag="vpall")
            at_all = cp.tile([128, S], BF16, tag="atall")
            tri = cp.tile([128, 128], BF16, tag="tri")
            wo_all = cp.tile([128, D], BF16, tag="wo")
            wq_all = cp.tile([128, D], F16, tag="wqa")
            wk_all = cp.tile([128, D], F16, tag="wka")
            wv_all = cp.tile([128, D], F16, tag="wva")

            nc.sync.dma_start(wk_all[:], wk_d[:])
            nc.sync.dma_start(wq_all[:], wq_d[:])
            nc.sync.dma_start(wv_all[:], wv_d[:])
            nc.sync.dma_start(wo_all[:], wot_d[:])
            nc.sync.dma_start(tri[:], tri_d[:])
            # ones columns of vp_all: cols {130t+64, 130t+129}
            nc.vector.memset(
                vp_all[:].rearrange("p (t x) -> p t x", x=VPW)[:, :, 64:VPW:65], 1.0)

            wq_t = [wq_all[:, i * 128:(i + 1) * 128] for i in range(6)]
            wk_t = [wk_all[:, i * 128:(i + 1) * 128] for i in range(6)]
            wv_t = [wv_all[:, i * 128:(i + 1) * 128] for i in range(6)]

            sched = [(qb, [0] + ([1] if qb in bset else [])) for qb in range(NQB)]

            # ---------------- projection + V -----------------------------
            def proj_chunk(c):
                c0, c1 = c * CH, (c + 1) * CH
                xc_all = xcp.tile([128, 6 * CH], F16, tag="xc", name=f"xca{c}")
                nc.sync.dma_start(
                    xc_all[:].rearrange("p (i c) -> p i c", c=CH),
                    xt_d[:, c0:c1].rearrange("(i p) c -> p i c", p=128))
                cosf_c = rtp.tile([128, CH], F32, tag="cosc", name=f"cosc{c}")
                sins_c = rtp.tile([128, CH], F32, tag="sinc", name=f"sinc{c}")
                nc.sync.dma_start(cosf_c[:], cosf_d[:, c0:c1])
                nc.sync.dma_start(sins_c[:], sins_d[:, c0:c1])
                xc = [xc_all[:, i * CH:(i + 1) * CH] for i in range(6)]
                psk = psP.tile([128, CH], F32, tag="pp", name=f"pk{c}")
                psq = psP.tile([128, CH], F32, tag="pp", name=f"pq{c}")
                for ps, w in ((psk, wk_t), (psq, wq_t)):
                    for i in range(6):
                        nc.tensor.matmul(ps[:], w[i], xc[i],
                                         start=(i == 0), stop=(i == 5))
                # gpsimd (Pool) cannot touch PSUM: ops reading psk/psq go on
                # DVE, SBUF-only ops (m2 mult, final add) go on Pool.
                for ps, dst in ((psk, kt_rot), (psq, qt_rot)):
                    tsw = rtp.tile([128, CH], F32, tag="tsw")
                    nc.vector.stream_shuffle(tsw[:], ps[:], ROT16)
                    m1 = rtp.tile([128, CH], F32, tag="m1")
                    nc.vector.tensor_tensor(m1[:], ps[:], cosf_c[:],
                                            op=AluOpType.mult)
                    m2 = rtp.tile([128, CH], F32, tag="m2")
                    nc.gpsimd.tensor_tensor(m2[:], tsw[:], sins_c[:],
                                            op=AluOpType.mult)
                    nc.gpsimd.tensor_tensor(dst[:, c0:c1], m1[:], m2[:],
                                            op=AluOpType.add)
                # V direct: [token, dh] = (x^T tile)^T @ Wv^T tile, f32 PSUM
                psv = psP.tile([128, CH], F32, tag="pp", name=f"pv{c}")
                for tt in range(4):
                    o0 = tt * 128
                    for i in range(6):
                        nc.tensor.matmul(
                            psv[:, o0:o0 + 128],
                            xc_all[:, i * CH + o0:i * CH + o0 + 128],
                            wv_t[i], start=(i == 0), stop=(i == 5))
                for tt in range(4):
                    t = 4 * c + tt
                    nc.vector.tensor_copy(
                        vp_all[:, t * VPW:(t + 1) * VPW].rearrange(
                            "p (h x) -> p h x", x=65)[:, :, 0:64],
                        psv[:, tt * 128:(tt + 1) * 128].rearrange(
                            "p (h x) -> p h x", x=64))

            # ---------------- attention ---------------------------------
            oproj_pend = []     # chunks whose at_all cols are complete

            def emit_oproj_one():
                if not oproj_pend:
                    return
                c, mt = oproj_pend[0]
                contr = 128 if c in bset else 64
                po = psP.tile([128, CH], F32, tag="pp", name=f"po{c}_{mt}")
                nc.tensor.matmul(
                    po[:], wo_all[0:contr, mt * 128:(mt + 1) * 128],
                    at_all[0:contr, c * CH:(c + 1) * CH],
                    start=True, stop=True)
                ot = osb.tile([128, CH], F16, tag="ot", name=f"ot{c}_{mt}")
                nc.vector.tensor_copy(ot[:], po[:])
                nc.sync.dma_start(
                    opart_d[mt * 128:(mt + 1) * 128, c * CH:(c + 1) * CH], ot[:])
                oproj_pend.pop(0)

            def attention_qb(qb, slots):
                nkt = 4 * (qb + 1)
                av = psAV.tile([128, 772], F32, tag="av", name=f"av{qb}")

                # PSUM zero regions are bank-granular (2KB): exactly one
                # start (first matmul into the slot's bank) and one stop
                # (last matmul, t == nkt-1 which only emits j == 3).  The
                # other chunks' first writes land on pending-zero bytes and
                # start fresh automatically.
                def emit_avs(exs_, ta_):
                    for s_ in slots:
                        for h_ in range(2):
                            t_ = ta_ + h_
                            m_ = t_ - 4 * qb
                            for j_ in range(max(0, m_), 4):
                                b_ = AVB[s_][j_]
                                nc.tensor.matmul(
                                    av[:, b_:b_ + 65],
                                    exs_[s_][:, h_ * QB + j_ * 128:
                                             h_ * QB + (j_ + 1) * 128],
                                    vp_all[:, t_ * VPW + s_ * 65:
                                           t_ * VPW + s_ * 65 + 65],
                                    start=(t_ == 0 and j_ == 0),
                                    stop=(t_ == nkt - 1))

                prev = None
                for g in range(nkt // 2):
                    ta = 2 * g
                    exs = {}
                    for s in slots:
                        lg = psL.tile([128, 2 * QB], F32, tag="lg",
                                      name=f"lg{qb}_{g}_{s}")
                        ex = exp_pool.tile([128, 2 * QB], BF16, tag="ex",
                                           name=f"ex{qb}_{g}_{s}")
                        start_col = 0
                        masks = []
                        for h in range(2):
                            t = ta + h
                            m = t - 4 * qb
                            off = 128 * m if m >= 0 else 0
                            base = h * QB
                            if h == 0:
                                start_col = off
                            nc.tensor.matmul(
                                lg[:, base + off:base + QB],
                                kt_rot[s * 64:(s + 1) * 64, t * KT:(t + 1) * KT],
                                qt_rot[s * 64:(s + 1) * 64,
                                       qb * QB + off:(qb + 1) * QB],
                                start=True, stop=True)
                            if m >= 0:
                                masks.append(base + off)
                        nc.scalar.activation(ex[:, start_col:2 * QB],
                                             lg[:, start_col:2 * QB],
                                             AF.Exp, scale=0.125)
                        for mb in masks:
                            nc.gpsimd.tensor_tensor(
                                ex[:, mb:mb + 128], ex[:, mb:mb + 128],
                                tri[:], op=AluOpType.mult)
                        exs[s] = ex
                    if prev is not None:
                        emit_avs(*prev)
                        emit_oproj_one()
                    prev = (exs, ta)
                emit_avs(*prev)

                # normalize + transpose into at_all
                rc = rcp.tile([128, 8], F32, tag="rc", name=f"rc{qb}")
                for s in slots:
                    sums = av[:, AVB[s][0]:AVB[s][0] + 260].rearrange(
                        "p (j x) -> p j x", x=65)[:, :, 64]
                    nc.vector.reciprocal(rc[:, 4 * s:4 * s + 4], sums)
                if DEBUG and qb == 1:
                    dbg_av_sb = cp.tile([128, 772], F32, tag="dav")
                    nc.vector.tensor_copy(dbg_av_sb[:], av[:])
                    nc.sync.dma_start(dbg_av[:], dbg_av_sb[:])
                for j in range(4):
                    stage = stp.tile([128, 128], BF16, tag="stg",
                                     name=f"st{qb}_{j}")
                    for s in slots:
                        nc.vector.tensor_scalar_mul(
                            stage[:, s * 64:(s + 1) * 64],
                            av[:, AVB[s][j]:AVB[s][j] + 64],
                            rc[:, 4 * s + j:4 * s + j + 1])
                    if DEBUG and qb == 1:
                        dbg_st_sb = cp.tile([128, 512], F32, tag="dst")
                        nc.vector.tensor_copy(
                            dbg_st_sb[:, j * 128:(j + 1) * 128], stage[:])
                        if j == 3:
                            nc.sync.dma_start(dbg_st[:], dbg_st_sb[:])
                    nc.sync.dma_start_transpose(
                        at_all[:, qb * QB + j * 128:qb * QB + (j + 1) * 128],
                        stage[:])
                for mt in range(6):
                    oproj_pend.append((qb, mt))

            # ---------------- interleaved main loop ---------------------
            proj_chunk(0)
            for c in range(1, NCH):
                proj_chunk(c)
                attention_qb(c - 1, sched[c - 1][1])
            attention_qb(7, sched[7][1])
            while oproj_pend:
                emit_oproj_one()

            if DEBUG:
                dbg_kt_sb = cp.tile([128, S], F32, tag="dkt")
                dbg_qt_sb = cp.tile([128, S], F32, tag="dqt")
                dbg_vp_sb = cp.tile([128, NKT * VPW], F32, tag="dvp")
                dbg_at_sb = cp.tile([128, S], F32, tag="dat")
                nc.vector.tensor_copy(dbg_kt_sb[:], kt_rot[:])
                nc.vector.tensor_copy(dbg_qt_sb[:], qt_rot[:])
                nc.vector.tensor_copy(dbg_vp_sb[:], vp_all[:])
                nc.vector.tensor_copy(dbg_at_sb[:], at_all[:])
                nc.sync.dma_start(dbg_kt[:], dbg_kt_sb[:])
                nc.sync.dma_start(dbg_qt[:], dbg_qt_sb[:])
                nc.sync.dma_start(dbg_vp[:], dbg_vp_sb[:])
                nc.sync.dma_start(dbg_at[:], dbg_at_sb[:])

    nc.compile()
    return nc


def _get_program(bset):
    key = tuple(bset)
    if key not in _PROGRAMS:
        _PROGRAMS[key] = _build_program(key)
    return _PROGRAMS[key]


def _prep_core_inputs(core, x2d_T16, token_positions, Wq, Wk, Wv, Wo):
    hA, hB = CORE_HEADS[core]
    pos = token_positions.astype(np.float64)
    inv_freq = 1.0 / (THETA ** (np.arange(0, DH, 2, dtype=np.float64) / DH))  # [32]
    ang = pos[:, None] * inv_freq[None, :]          # [S, 32]
    cosv, sinv = np.cos(ang), np.sin(ang)           # [S, 32]

    cosf = np.empty((128, S), np.float32)
    sins = np.empty((128, S), np.float32)
    for r in range(64):
        q, i = r // 32, r % 32
        f = 16 * q + (i % 16)
        cosf[r] = cosf[r + 64] = cosv[:, f].astype(np.float32)
        sgn = -1.0 if i < 16 else 1.0
        sins[r] = sins[r + 64] = (sgn * sinv[:, f]).astype(np.float32)

    rows = np.concatenate([hA * DH + PERM64, hB * DH + PERM64])
    # weight tiles pre-arranged as [128, 6*128]: [p, i*128+c] = W[rows[c], i*128+p]
    def warr(W, rr):
        wt = W[rr].T.astype(np.float16)               # [768, 128]
        return np.ascontiguousarray(
            wt.reshape(6, 128, 128).transpose(1, 0, 2).reshape(128, 768))

    vrows = np.concatenate([np.arange(hA * DH, (hA + 1) * DH),
                            np.arange(hB * DH, (hB + 1) * DH)])
    wq = warr(Wq, rows)
    wk = warr(Wk, rows)
    wv = warr(Wv, vrows)
    wot = np.ascontiguousarray(Wo[:, vrows].T).astype(np.float32)  # [128,768]

    tri = np.where(np.arange(128)[None, :] >= np.arange(128)[:, None],
                   1.0, 0.0)  # [k', q'] 0/1 mask

    def bf16(a):
        import ml_dtypes
        return np.asarray(a).astype(ml_dtypes.bfloat16)

    return {
        "xt": x2d_T16,
        "wq": wq, "wk": wk, "wv": wv, "wot": bf16(wot),
        "cosf": cosf, "sins": sins,
        "tri": bf16(tri),
    }


def _dispatch_group(nc, in_maps, devices):
    """Async-dispatch one program on a device subset; returns (arrs, names, avals, n)."""
    import jax
    from jax.sharding import Mesh, PartitionSpec
    from concourse import bass2jax, mybir

    bass2jax.install_neuronx_cc_hook()
    n = len(in_maps)
    partition_name = (nc.partition_id_tensor.name
                      if nc.partition_id_tensor else None)
    in_names, out_names, out_avals, zero_outs = [], [], [], []
    for alloc in nc.m.functions[0].allocations:
        if not isinstance(alloc, mybir.MemoryLocationSet):
            continue
        name = alloc.memorylocations[0].name
        if alloc.kind == "ExternalInput":
            if name != partition_name:
                in_names.append(name)
        elif alloc.kind == "ExternalOutput":
            shape = tuple(alloc.tensor_shape)
            dtype = mybir.dt.np(alloc.dtype)
            out_names.append(name)
            out_avals.append(jax.core.ShapedArray(shape, dtype))
            zero_outs.append(np.zeros(shape, dtype))
    n_params = len(in_names)
    all_names = in_names + out_names
    if partition_name is not None:
        all_names = all_names + [partition_name]
    donate = tuple(range(n_params, n_params + len(out_names)))

    def _body(*args):
        operands = list(args)
        if partition_name is not None:
            operands.append(bass2jax.partition_id_tensor())
        outs = bass2jax._bass_exec_p.bind(
            *operands, out_avals=tuple(out_avals), in_names=tuple(all_names),
            out_names=tuple(out_names), lowering_input_output_aliases=(),
            sim_require_finite=True, sim_require_nnan=True, nc=nc)
        return tuple(outs)

    try:
        from jax.experimental.shard_map import shard_map
    except ImportError:
        from jax.shard_map import shard_map  # newer jax

    mesh = Mesh(np.asarray(devices), ("core",))
    in_specs = (PartitionSpec("core"),) * (n_params + len(out_names))
    out_specs = (PartitionSpec("core"),) * len(out_names)
    sharded = jax.jit(
        shard_map(_body, mesh=mesh, in_specs=in_specs, out_specs=out_specs,
                  check_rep=False),
        donate_argnums=donate, keep_unused=True)
    per_core = [[np.asarray(m[nm]) for nm in in_names] for m in in_maps]
    concat_in = [np.concatenate([per_core[c][i] for c in range(n)], axis=0)
                 for i in range(n_params)]
    concat_zeros = [np.zeros((n * z.shape[0], *z.shape[1:]), z.dtype)
                    for z in zero_outs]
    out_arrs = sharded(*concat_in, *concat_zeros)
    return out_arrs, out_names, out_avals, n


def kernel(x, token_positions, Wq, Wk, Wv, Wo):
    import jax

    x = np.asarray(x)
    token_positions = np.asarray(token_positions)
    Wq, Wk, Wv, Wo = (np.asarray(a, np.float32) for a in (Wq, Wk, Wv, Wo))
    B = x.shape[0]
    assert x.shape == (B, S, D) and B == 1

    x2d_T16 = np.ascontiguousarray(x[0].T.astype(np.float16))  # [768, 4096]

    in_maps = [_prep_core_inputs(c, x2d_T16, token_positions, Wq, Wk, Wv, Wo)
               for c in range(8)]

    nc_even = _get_program(BSET_EVEN)
    nc_odd = _get_program(BSET_ODD)

    devs = jax.devices()
    # even program on devices 0-3 <- logical cores 0,2,4,6
    # odd  program on devices 4-7 <- logical cores 1,3,5,7
    g1_maps = [in_maps[c] for c in (0, 2, 4, 6)]
    g2_maps = [in_maps[c] for c in (1, 3, 5, 7)]

    arrs1, names1, avals1, n1 = _dispatch_group(nc_even, g1_maps, devs[0:4])
    arrs2, names2, avals2, n2 = _dispatch_group(nc_odd, g2_maps, devs[4:8])

    def collect(arrs, names, avals, n):
        res = []
        for c in range(n):
            res.append({
                nm: np.asarray(arrs[i]).reshape(n, *avals[i].shape)[c]
                for i, nm in enumerate(names)})
        return res

    res1 = collect(arrs1, names1, avals1, n1)
    res2 = collect(arrs2, names2, avals2, n2)

    acc = np.zeros((D, S), np.float32)
    for r in res1 + res2:
        acc += r["opart"].astype(np.float32)
    out = np.ascontiguousarray(acc.T).reshape(1, S, D)
    return out


# revision 23
# speedup vs baseline: 1.0792x; 1.0792x over previous
"""Multi-head causal self-attention with RoPE on 8 Trainium2 NeuronCores.

Sharding: 12 heads over 8 cores. Core pairs (2p, 2p+1) share 3 heads:
  core 2p:   slot A = head 3p   (all 8 q-blocks), slot B = head 3p+1, q-blocks {0,1,6,7}
  core 2p+1: slot A = head 3p+2 (all 8 q-blocks), slot B = head 3p+1, q-blocks {2,3,4,5}
Every core: 2 heads on 128 partitions, 12 (head, q-block) pairs, identical causal cost.
Even/odd cores need different programs (q-block sets differ) -> two NEFFs dispatched
concurrently on device groups [0..3] / [4..7].

Pipeline (per core):
  x^T (f16) -> Q^T/K^T proj (f16 matmul, f32 PSUM) -> RoPE (DVE/Pool) -> f16 Q^T/K^T
  V computed directly as [token, dh] (x tiles stationary) -> bf16 [V_A|1|V_B|1] tiles
  logits^T = K_tile @ Q^T (f16) -> f32 PSUM -> exp (Act, scale=1/8) -> bf16 scores
  AV flipped: scores as stationary [keys,128q], moving [V_s|1] (65 cols) ->
    psAV [q, 4*(64+1)] f32 accumulated over key tiles; sums in col 64 of each chunk
  normalize: reciprocal (DVE) + per-partition tensor_scalar (Pool) -> bf16 A [q, dh]
  A^T via DMA xbar transpose [128,128] -> at_all bf16 [2*dh, S]
  O-proj: wo (bf16) x at_all -> f32 PSUM -> f16 SBUF -> DMA partials; host sums.

RoPE pairs are (d, d+16) within each 32-partition quadrant (folded into a host-side
weight-row permutation) so the pair swap is one stream_shuffle rotate-16.
"""
import sys, os
sys.path.insert(0, "/opt/trn_rl_repo")
os.environ.setdefault("MYCRO_LOCAL_CACHE", "1")

import numpy as np

S, D, H, DH = 4096, 768, 12, 64
NCH, CH = 8, 512     # token chunks (projection phase)
NQB, QB = 8, 512     # query blocks
NKT, KT = 32, 128    # key tiles
VPW = 130            # vp_all per-ktile width: [V_A(64) | 1 | V_B(64) | 1]
THETA = 10000.0
ROT16 = [(i + 16) % 32 for i in range(32)]

BSET_EVEN = (0, 1, 6, 7)
BSET_ODD = (2, 3, 4, 5)

# core -> (headA, headB)
CORE_HEADS = []
for p in range(4):
    CORE_HEADS.append((3 * p, 3 * p + 1))
    CORE_HEADS.append((3 * p + 2, 3 * p + 1))

# row r (0..63) inside a head slot -> original within-head dim.
# quadrant q = r//32, i = r%32: freq f = 16*q + (i%16); i<16 -> dim 2f, else 2f+1.
PERM64 = []
for r in range(64):
    q, i = r // 32, r % 32
    f = 16 * q + (i % 16)
    PERM64.append(2 * f if i < 16 else 2 * f + 1)
PERM64 = np.array(PERM64)

# psAV chunk base offsets: slot 0 chunks at 65*j in bank 0, slot 1 at 512+65*j
# in bank 1 (each 65-wide [A(64)|sum] region must stay within one 2KB bank).
AVB = [[65 * j for j in range(4)], [512 + 65 * j for j in range(4)]]

_PROGRAMS = {}


def _build_program(bset):
    import concourse.bass as bass
    import concourse.tile as tile
    from concourse import bacc, mybir
    from concourse.alu_op_type import AluOpType

    dt = mybir.dt
    F32, F16, BF16 = dt.float32, dt.float16, dt.bfloat16
    AF = mybir.ActivationFunctionType

    nc = bacc.Bacc("TRN2", target_bir_lowering=False, debug=False, num_devices=4)

    xt_d = nc.dram_tensor("xt", [D, S], F16, kind="ExternalInput").ap()
    wq_d = nc.dram_tensor("wq", [128, D], F16, kind="ExternalInput").ap()
    wk_d = nc.dram_tensor("wk", [128, D], F16, kind="ExternalInput").ap()
    wv_d = nc.dram_tensor("wv", [128, D], F16, kind="ExternalInput").ap()
    wot_d = nc.dram_tensor("wot", [128, D], BF16, kind="ExternalInput").ap()
    cosf_d = nc.dram_tensor("cosf", [128, S], F32, kind="ExternalInput").ap()
    sins_d = nc.dram_tensor("sins", [128, S], F32, kind="ExternalInput").ap()
    eyem_d = nc.dram_tensor("eyem", [128, 128], F16, kind="ExternalInput").ap()
    maskc_d = nc.dram_tensor("maskc", [128, 128], F16, kind="ExternalInput").ap()
    opart_d = nc.dram_tensor("opart", [D, S], F16, kind="ExternalOutput").ap()
    DEBUG = bool(os.environ.get("MHA_DEBUG"))
    if DEBUG:
        dbg_kt = nc.dram_tensor("dbg_kt", [128, S], F32, kind="ExternalOutput").ap()
        dbg_qt = nc.dram_tensor("dbg_qt", [128, S], F32, kind="ExternalOutput").ap()
        dbg_vp = nc.dram_tensor("dbg_vp", [128, NKT * VPW], F32,
                                kind="ExternalOutput").ap()
        dbg_at = nc.dram_tensor("dbg_at", [128, S], F32, kind="ExternalOutput").ap()
        dbg_av = nc.dram_tensor("dbg_av", [128, 772], F32, kind="ExternalOutput").ap()
        dbg_st = nc.dram_tensor("dbg_st", [128, 512], F32, kind="ExternalOutput").ap()

    with tile.TileContext(nc) as tc:
        with (
            tc.tile_pool(name="const", bufs=1) as cp,
            tc.tile_pool(name="xc", bufs=4) as xcp,
            tc.tile_pool(name="cs", bufs=4) as csp,
            tc.tile_pool(name="rt", bufs=3) as rtp,
            tc.tile_pool(name="ex", bufs=6) as exp_pool,
            tc.tile_pool(name="stg", bufs=4) as stp,
            tc.tile_pool(name="rcp", bufs=2) as rcp,
            tc.tile_pool(name="osb", bufs=4) as osb,
            tc.tile_pool(name="psP", bufs=2, space="PSUM") as psP,
            tc.tile_pool(name="psL", bufs=2, space="PSUM") as psL,
            tc.tile_pool(name="psAV", bufs=1, space="PSUM") as psAV,
        ):
            kt_rot = cp.tile([128, S], F16, tag="ktrot")
            qt_rot = cp.tile([128, S], F16, tag="qtrot")
            vp_all = cp.tile([128, NKT * VPW], BF16, tag="vpall")
            at_all = cp.tile([128, S], BF16, tag="atall")
            eyem = cp.tile([128, 128], F16, tag="eyem")
            maskc = cp.tile([128, 128], F16, tag="maskc")
            wo_all = cp.tile([128, D], BF16, tag="wo")
            wq_all = cp.tile([128, D], F16, tag="wqa")
            wk_all = cp.tile([128, D], F16, tag="wka")
            wv_all = cp.tile([128, D], F16, tag="wva")

            # All DMAs on the SP queue; loads are prefetched deep (bufs=4
            # pools) so they never hold the in-order queue on a WAR wait.
            # The Activation queue carries no DMAs: exp dispatch never blocks.
            nc.sync.dma_start(wk_all[:], wk_d[:])
            nc.sync.dma_start(wq_all[:], wq_d[:])
            nc.sync.dma_start(wv_all[:], wv_d[:])
            nc.sync.dma_start(wo_all[:], wot_d[:])
            nc.sync.dma_start(eyem[:], eyem_d[:])
            nc.sync.dma_start(maskc[:], maskc_d[:])
            # ones columns of vp_all: cols {130t+64, 130t+129}
            nc.vector.memset(
                vp_all[:].rearrange("p (t x) -> p t x", x=VPW)[:, :, 64:VPW:65], 1.0)

            wq_t = [wq_all[:, i * 128:(i + 1) * 128] for i in range(6)]
            wk_t = [wk_all[:, i * 128:(i + 1) * 128] for i in range(6)]
            wv_t = [wv_all[:, i * 128:(i + 1) * 128] for i in range(6)]

            # ---------------- projection + V -----------------------------
            def proj_chunk(c):
                c0, c1 = c * CH, (c + 1) * CH
                xc_all = xcp.tile([128, 6 * CH], F16, tag="xc", name=f"xca{c}")
                nc.sync.dma_start(
                    xc_all[:].rearrange("p (i c) -> p i c", c=CH),
                    xt_d[:, c0:c1].rearrange("(i p) c -> p i c", p=128))
                cosf_c = csp.tile([128, CH], F32, tag="cosc", name=f"cosc{c}")
                sins_c = csp.tile([128, CH], F32, tag="sinc", name=f"sinc{c}")
                nc.sync.dma_start(cosf_c[:], cosf_d[:, c0:c1])
                nc.sync.dma_start(sins_c[:], sins_d[:, c0:c1])
                xc = [xc_all[:, i * CH:(i + 1) * CH] for i in range(6)]
                psk = psP.tile([128, CH], F32, tag="pp", name=f"pk{c}")
                psq = psP.tile([128, CH], F32, tag="pp", name=f"pq{c}")
                for ps, w in ((psk, wk_t), (psq, wq_t)):
                    for i in range(6):
                        nc.tensor.matmul(ps[:], w[i], xc[i],
                                         start=(i == 0), stop=(i == 5))
                # gpsimd (Pool) cannot touch PSUM: ops reading psk/psq go on
                # DVE, SBUF-only ops (m2 mult, final add) go on Pool.
                for ps, dst in ((psk, kt_rot), (psq, qt_rot)):
                    tsw = rtp.tile([128, CH], F32, tag="tsw")
                    nc.vector.stream_shuffle(tsw[:], ps[:], ROT16)
                    m1 = rtp.tile([128, CH], F32, tag="m1")
                    nc.vector.tensor_tensor(m1[:], ps[:], cosf_c[:],
                                            op=AluOpType.mult)
                    m2 = rtp.tile([128, CH], F32, tag="m2")
                    nc.gpsimd.tensor_tensor(m2[:], tsw[:], sins_c[:],
                                            op=AluOpType.mult)
                    nc.gpsimd.tensor_tensor(dst[:, c0:c1], m1[:], m2[:],
                                            op=AluOpType.add)
                # V direct: [token, dh] = (x^T tile)^T @ Wv^T tile, f32 PSUM
                psv = psP.tile([128, CH], F32, tag="pp", name=f"pv{c}")
                for tt in range(4):
                    o0 = tt * 128
                    for i in range(6):
                        nc.tensor.matmul(
                            psv[:, o0:o0 + 128],
                            xc_all[:, i * CH + o0:i * CH + o0 + 128],
                            wv_t[i], start=(i == 0), stop=(i == 5))
                for tt in range(4):
                    t = 4 * c + tt
                    nc.vector.tensor_copy(
                        vp_all[:, t * VPW:(t + 1) * VPW].rearrange(
                            "p (h x) -> p h x", x=65)[:, :, 0:64],
                        psv[:, tt * 128:(tt + 1) * 128].rearrange(
                            "p (h x) -> p h x", x=64))

            # ---------------- attention ---------------------------------
            oproj_pend = []     # chunks whose at_all cols are complete

            def emit_oproj_one():
                if not oproj_pend:
                    return
                c, mt = oproj_pend[0]
                contr = 128 if c in bset else 64
                po = psP.tile([128, CH], F32, tag="pp", name=f"po{c}_{mt}")
                nc.tensor.matmul(
                    po[:], wo_all[0:contr, mt * 128:(mt + 1) * 128],
                    at_all[0:contr, c * CH:(c + 1) * CH],
                    start=True, stop=True)
                ot = osb.tile([128, CH], F16, tag="ot", name=f"ot{c}_{mt}")
                nc.vector.tensor_copy(ot[:], po[:])
                nc.sync.dma_start(
                    opart_d[mt * 128:(mt + 1) * 128, c * CH:(c + 1) * CH], ot[:])
                oproj_pend.pop(0)

            # Attention runs as a CALL of up to two concurrent LANES
            # (qb, slot): a 2-slot q-block is two lanes of the same qb;
            # two single-slot q-blocks are paired into one call so both
            # psAV banks and the lg double-buffer stay filled (keeps the
            # exp stream dense for the Activation engine).
            def attention_call(lanes):
                ngr = [2 * (qb + 1) for qb, _ in lanes]
                av = psAV.tile([128, 772], F32,
                               tag="av", name=f"av{lanes[0][0]}")
                rc = rcp.tile([128, 8], F32, tag="rc", name=f"rc{lanes[0][0]}")
                qb_rem = {}
                for qb, _ in lanes:
                    qb_rem[qb] = qb_rem.get(qb, 0) + 1

                # PSUM zero regions are bank-granular (2KB): one start
                # (first matmul into the lane's bank) and one stop (last
                # matmul, t == nkt-1 which only emits j == 3).  Other
                # chunks' first writes land on pending-zero bytes and
                # start fresh automatically.
                def emit_avs(li_, exs_, ta_):
                    qb_, s_ = lanes[li_]
                    nkt_ = 4 * (qb_ + 1)
                    for h_ in range(2):
                        t_ = ta_ + h_
                        m_ = t_ - 4 * qb_
                        for j_ in range(max(0, m_), 4):
                            b_ = AVB[li_][j_]
                            nc.tensor.matmul(
                                av[:, b_:b_ + 65],
                                exs_[:, h_ * QB + j_ * 128:
                                     h_ * QB + (j_ + 1) * 128],
                                vp_all[:, t_ * VPW + s_ * 65:
                                       t_ * VPW + s_ * 65 + 65],
                                start=(t_ == 0 and j_ == 0),
                                stop=(t_ == nkt_ - 1))

                def drain_qb(qb_):
                    linfo = [(li_, s_) for li_, (q_, s_) in enumerate(lanes)
                             if q_ == qb_]
                    for li_, s_ in linfo:
                        sums = av[:, AVB[li_][0]:AVB[li_][0] + 260].rearrange(
                            "p (j x) -> p j x", x=65)[:, :, 64]
                        nc.vector.reciprocal(rc[:, 4 * li_:4 * li_ + 4], sums)
                    if DEBUG and qb_ == 1:
                        dbg_av_sb = cp.tile([128, 772], F32, tag="dav")
                        nc.vector.tensor_copy(dbg_av_sb[:], av[:])
                        nc.sync.dma_start(dbg_av[:], dbg_av_sb[:])
                    for j in range(4):
                        stage = stp.tile([128, 128], BF16, tag="stg",
                                         name=f"st{qb_}_{j}")
                        for li_, s_ in linfo:
                            nc.vector.tensor_scalar_mul(
                                stage[:, s_ * 64:(s_ + 1) * 64],
                                av[:, AVB[li_][j]:AVB[li_][j] + 64],
                                rc[:, 4 * li_ + j:4 * li_ + j + 1])
                        if DEBUG and qb_ == 1:
                            dbg_st_sb = cp.tile([128, 512], F32, tag="dst")
                            nc.vector.tensor_copy(
                                dbg_st_sb[:, j * 128:(j + 1) * 128], stage[:])
                            if j == 3:
                                nc.sync.dma_start(dbg_st[:], dbg_st_sb[:])
                        nc.sync.dma_start_transpose(
                            at_all[:, qb_ * QB + j * 128:
                                   qb_ * QB + (j + 1) * 128],
                            stage[:])
                    for mt in range(6):
                        oproj_pend.append((qb_, mt))

                prev = {}
                for g in range(max(ngr)):
                    for li, (qb, s) in enumerate(lanes):
                        if g >= ngr[li]:
                            continue
                        ta = 2 * g
                        lg = psL.tile([128, 2 * QB], F32, tag="lg",
                                      name=f"lg{qb}_{g}_{s}")
                        ex = exp_pool.tile([128, 2 * QB], BF16, tag="ex",
                                           name=f"ex{qb}_{g}_{s}")
                        start_col = 0
                        for h in range(2):
                            t = ta + h
                            m = t - 4 * qb
                            off = 128 * m if m >= 0 else 0
                            base = h * QB
                            if h == 0:
                                start_col = off
                            diag = m >= 0
                            nc.tensor.matmul(
                                lg[:, base + off:base + QB],
                                kt_rot[s * 64:(s + 1) * 64, t * KT:(t + 1) * KT],
                                qt_rot[s * 64:(s + 1) * 64,
                                       qb * QB + off:(qb + 1) * QB],
                                start=True, stop=not diag)
                            if diag:
                                # causal mask folded into the logits via a
                                # second accumulating matmul: eye^T @ maskc
                                # adds -60000 above the diagonal.
                                nc.tensor.matmul(
                                    lg[:, base + off:base + off + 128],
                                    eyem[:], maskc[:],
                                    start=False, stop=True)
                        nc.scalar.activation(ex[:, start_col:2 * QB],
                                             lg[:, start_col:2 * QB],
                                             AF.Exp, scale=0.125)
                        if li in prev:
                            emit_avs(li, *prev[li])
                            if g >= 2:
                                emit_oproj_one()
                        if g == ngr[li] - 1:
                            emit_avs(li, ex, ta)
                            del_qb = lanes[li][0]
                            qb_rem[del_qb] -= 1
                            if qb_rem[del_qb] == 0:
                                drain_qb(del_qb)
                            prev.pop(li, None)
                        else:
                            prev[li] = (ex, ta)

            # ---------------- interleaved main loop ---------------------
            # calls: 2-slot q-blocks as (A,B); single-slot q-blocks paired.
            two = [qb for qb in range(NQB) if qb in bset]
            one = [qb for qb in range(NQB) if qb not in bset]
            calls = [[(qb, 0), (qb, 1)] for qb in two]
            calls += [[(qb, 0) for qb in one[i:i + 2]]
                      for i in range(0, len(one), 2)]
            calls.sort(key=lambda L: max(q for q, _ in L))

            next_c = 0
            for call in calls:
                req = max(q for q, _ in call)
                while next_c <= req:
                    proj_chunk(next_c)
                    next_c += 1
                attention_call(call)
            while oproj_pend:
                emit_oproj_one()

            if DEBUG:
                dbg_kt_sb = cp.tile([128, S], F32, tag="dkt")
                dbg_qt_sb = cp.tile([128, S], F32, tag="dqt")
                dbg_vp_sb = cp.tile([128, NKT * VPW], F32, tag="dvp")
                dbg_at_sb = cp.tile([128, S], F32, tag="dat")
                nc.vector.tensor_copy(dbg_kt_sb[:], kt_rot[:])
                nc.vector.tensor_copy(dbg_qt_sb[:], qt_rot[:])
                nc.vector.tensor_copy(dbg_vp_sb[:], vp_all[:])
                nc.vector.tensor_copy(dbg_at_sb[:], at_all[:])
                nc.sync.dma_start(dbg_kt[:], dbg_kt_sb[:])
                nc.sync.dma_start(dbg_qt[:], dbg_qt_sb[:])
                nc.sync.dma_start(dbg_vp[:], dbg_vp_sb[:])
                nc.sync.dma_start(dbg_at[:], dbg_at_sb[:])

    nc.compile()
    return nc


def _get_program(bset):
    key = tuple(bset)
    if key not in _PROGRAMS:
        _PROGRAMS[key] = _build_program(key)
    return _PROGRAMS[key]


def _prep_core_inputs(core, x2d_T16, token_positions, Wq, Wk, Wv, Wo):
    hA, hB = CORE_HEADS[core]
    pos = token_positions.astype(np.float64)
    inv_freq = 1.0 / (THETA ** (np.arange(0, DH, 2, dtype=np.float64) / DH))  # [32]
    ang = pos[:, None] * inv_freq[None, :]          # [S, 32]
    cosv, sinv = np.cos(ang), np.sin(ang)           # [S, 32]

    cosf = np.empty((128, S), np.float32)
    sins = np.empty((128, S), np.float32)
    for r in range(64):
        q, i = r // 32, r % 32
        f = 16 * q + (i % 16)
        cosf[r] = cosf[r + 64] = cosv[:, f].astype(np.float32)
        sgn = -1.0 if i < 16 else 1.0
        sins[r] = sins[r + 64] = (sgn * sinv[:, f]).astype(np.float32)

    rows = np.concatenate([hA * DH + PERM64, hB * DH + PERM64])
    # weight tiles pre-arranged as [128, 6*128]: [p, i*128+c] = W[rows[c], i*128+p]
    def warr(W, rr):
        wt = W[rr].T.astype(np.float16)               # [768, 128]
        return np.ascontiguousarray(
            wt.reshape(6, 128, 128).transpose(1, 0, 2).reshape(128, 768))

    vrows = np.concatenate([np.arange(hA * DH, (hA + 1) * DH),
                            np.arange(hB * DH, (hB + 1) * DH)])
    wq = warr(Wq, rows)
    wk = warr(Wk, rows)
    wv = warr(Wv, vrows)
    wot = np.ascontiguousarray(Wo[:, vrows].T).astype(np.float32)  # [128,768]

    # [k', q'] additive causal mask for diagonal 128-tiles
    maskc = np.where(np.arange(128)[None, :] >= np.arange(128)[:, None],
                     0.0, -60000.0).astype(np.float16)

    def bf16(a):
        import ml_dtypes
        return np.asarray(a).astype(ml_dtypes.bfloat16)

    return {
        "xt": x2d_T16,
        "wq": wq, "wk": wk, "wv": wv, "wot": bf16(wot),
        "cosf": cosf, "sins": sins,
        "eyem": np.eye(128, dtype=np.float16), "maskc": maskc,
    }


def _dispatch_group(nc, in_maps, devices):
    """Async-dispatch one program on a device subset; returns (arrs, names, avals, n)."""
    import jax
    from jax.sharding import Mesh, PartitionSpec
    from concourse import bass2jax, mybir

    bass2jax.install_neuronx_cc_hook()
    n = len(in_maps)
    partition_name = (nc.partition_id_tensor.name
                      if nc.partition_id_tensor else None)
    in_names, out_names, out_avals, zero_outs = [], [], [], []
    for alloc in nc.m.functions[0].allocations:
        if not isinstance(alloc, mybir.MemoryLocationSet):
            continue
        name = alloc.memorylocations[0].name
        if alloc.kind == "ExternalInput":
            if name != partition_name:
                in_names.append(name)
        elif alloc.kind == "ExternalOutput":
            shape = tuple(alloc.tensor_shape)
            dtype = mybir.dt.np(alloc.dtype)
            out_names.append(name)
            out_avals.append(jax.core.ShapedArray(shape, dtype))
            zero_outs.append(np.zeros(shape, dtype))
    n_params = len(in_names)
    all_names = in_names + out_names
    if partition_name is not None:
        all_names = all_names + [partition_name]
    donate = tuple(range(n_params, n_params + len(out_names)))

    def _body(*args):
        operands = list(args)
        if partition_name is not None:
            operands.append(bass2jax.partition_id_tensor())
        outs = bass2jax._bass_exec_p.bind(
            *operands, out_avals=tuple(out_avals), in_names=tuple(all_names),
            out_names=tuple(out_names), lowering_input_output_aliases=(),
            sim_require_finite=True, sim_require_nnan=True, nc=nc)
        return tuple(outs)

    try:
        from jax.experimental.shard_map import shard_map
    except ImportError:
        from jax.shard_map import shard_map  # newer jax

    mesh = Mesh(np.asarray(devices), ("core",))
    in_specs = (PartitionSpec("core"),) * (n_params + len(out_names))
    out_specs = (PartitionSpec("core"),) * len(out_names)
    sharded = jax.jit(
        shard_map(_body, mesh=mesh, in_specs=in_specs, out_specs=out_specs,
                  check_rep=False),
        donate_argnums=donate, keep_unused=True)
    per_core = [[np.asarray(m[nm]) for nm in in_names] for m in in_maps]
    concat_in = [np.concatenate([per_core[c][i] for c in range(n)], axis=0)
                 for i in range(n_params)]
    concat_zeros = [np.zeros((n * z.shape[0], *z.shape[1:]), z.dtype)
                    for z in zero_outs]
    out_arrs = sharded(*concat_in, *concat_zeros)
    return out_arrs, out_names, out_avals, n


def kernel(x, token_positions, Wq, Wk, Wv, Wo):
    import jax

    x = np.asarray(x)
    token_positions = np.asarray(token_positions)
    Wq, Wk, Wv, Wo = (np.asarray(a, np.float32) for a in (Wq, Wk, Wv, Wo))
    B = x.shape[0]
    assert x.shape == (B, S, D) and B == 1

    x2d_T16 = np.ascontiguousarray(x[0].T.astype(np.float16))  # [768, 4096]

    in_maps = [_prep_core_inputs(c, x2d_T16, token_positions, Wq, Wk, Wv, Wo)
               for c in range(8)]

    nc_even = _get_program(BSET_EVEN)
    nc_odd = _get_program(BSET_ODD)

    devs = jax.devices()
    # even program on devices 0-3 <- logical cores 0,2,4,6
    # odd  program on devices 4-7 <- logical cores 1,3,5,7
    g1_maps = [in_maps[c] for c in (0, 2, 4, 6)]
    g2_maps = [in_maps[c] for c in (1, 3, 5, 7)]

    arrs1, names1, avals1, n1 = _dispatch_group(nc_even, g1_maps, devs[0:4])
    arrs2, names2, avals2, n2 = _dispatch_group(nc_odd, g2_maps, devs[4:8])

    def collect(arrs, names, avals, n):
        res = []
        for c in range(n):
            res.append({
                nm: np.asarray(arrs[i]).reshape(n, *avals[i].shape)[c]
                for i, nm in enumerate(names)})
        return res

    res1 = collect(arrs1, names1, avals1, n1)
    res2 = collect(arrs2, names2, avals2, n2)

    acc = np.zeros((D, S), np.float32)
    for r in res1 + res2:
        acc += r["opart"].astype(np.float32)
    out = np.ascontiguousarray(acc.T).reshape(1, S, D)
    return out
